# revision 1
# baseline (speedup 1.0000x reference)
"""MixedFFN Trainium2 kernel (8 NeuronCores, SPMD).

Problem: x [8, 2048, 1024]; shared FFN (W1S [2048,1024], W2S [1024,2048])
applied to positions 0..1984 of every batch; per-position FFN
(W1NS [64,1024,2048], W2NS [64,2048,1024]) applied to positions 1984..2048.
gelu is exact (erf). Output [8, 2048, 1024] fp32.

Sharding:
  - Shared part: data-parallel over batch. Core i computes the shared FFN
    for batch i over ALL 2048 positions (the last 64 are computed and
    discarded -- cheaper than a ragged tail) using replicated W1S/W2S.
  - Per-position part: sharded over positions. Core i handles positions
    1984+8i .. 1984+8(i+1) for ALL batches, so each NS weight byte is read
    from HBM exactly once across the chip.

Device kernel (per core, identical program, different data):
  - All matmul inputs are float32r (TF32-like full-rate PE mode, measured
    ~1.5e-4 rel err for K=1024 contractions).
  - MM1: H.T[f,r] = sum_dc W1T[dc,f-chunk].T @ X.T[dc, rows]; gelu on ACT.
  - MM2: Y[r,d] = sum_fc HT[fc, r-chunk].T @ W2T[fc, d]  (natural layout out).
  - NS MM1 packs the 4 f-blocks of h into 32-partition PSUM bands via
    tile_position col-groups (stationary is x_p.T [128,8]); NS MM2 packs 4
    positions into bands the same way. h.T comes from 4 PE transposes.
  - The ~128MB/core NS weight stream is the DMA bottleneck; its dc/fc-tile
    steps are Bresenham-interleaved with the shared-block steps so the PE
    and DMA engines stay concurrently busy; NS DMAs ride the ACT HWDGE ring,
    shared DMAs the SP ring.

Host side: shards/transposes inputs (numpy), feeds the SPMD run, reassembles.
"""

import os
import sys

import ml_dtypes
import numpy as np

BF16 = ml_dtypes.bfloat16


def _lp_np():
    return (
        np.float16
        if os.environ.get("MIXEDFFN_NS_DT", "fp16") == "fp16"
        else BF16
    )


def _ns1_np():
    return _lp_np() if os.environ.get("MIXEDFFN_NS", "mm2") in ("all", "mm1") else np.float32


def _ns2_np():
    return _lp_np() if os.environ.get("MIXEDFFN_NS", "mm2") in ("all", "mm2") else np.float32

for _p in ("/opt/trn_rl_repo",):
    if os.path.isdir(_p) and _p not in sys.path:
        sys.path.insert(0, _p)

B, T, D, F, LNS = 8, 2048, 1024, 2048, 64
S = T - LNS  # 1984
NCORES = 8
PPC = LNS // NCORES  # 8 positions per core
RB = 512  # shared row-block
NRB = T // RB  # 4
DC, FC = D // 128, F // 128  # 8, 16 k-chunks
FB = F // 512  # 4 f-blocks for NS h banding

LAST_RESULTS = None  # BassKernelResults of the most recent run (for test.py)

_cached = None


def _split_multiwaits(nc, mybir, max_waits=1):
    """The neuronxcc walrus on the axon compile path rejects >1 sem wait per
    instruction ("Too many sync wait commands"); hoist extras onto same-engine
    NoOps placed immediately before (per-engine streams execute in order)."""
    seq = 0
    for fn in nc.m.functions:
        for blk in fn.blocks:
            out, changed = [], False
            for inst in blk.instructions:
                si = inst.sync_info
                waits = list(si.on_wait) if si is not None else []
                if len(waits) > max_waits:
                    changed = True
                    for w in waits[:-max_waits]:
                        seq += 1
                        nop = mybir.InstNoOp(name=f"I-waitfix-{seq}", ins=[], outs=[])
                        nop.engine = inst.engine
                        nop.sync_info = mybir.SyncInfo(on_wait=[w], on_update=[])
                        out.append(nop)
                    inst.sync_info = mybir.SyncInfo(
                        on_wait=waits[-max_waits:], on_update=list(si.on_update)
                    )
                out.append(inst)
            if changed:
                blk.instructions = out


def _interleave(a, b):
    """Merge two step lists proportionally (Bresenham); each step is a
    zero-arg callable that emits instructions."""
    if not b:
        return list(a)
    if not a:
        return list(b)
    out = []
    ia = ib = 0
    na, nb = len(a), len(b)
    while ia < na or ib < nb:
        if ib * na <= ia * nb:
            if ib < nb:
                out.append(b[ib])
                ib += 1
            else:
                out.append(a[ia])
                ia += 1
        else:
            if ia < na:
                out.append(a[ia])
                ia += 1
            else:
                out.append(b[ib])
                ib += 1
    return out


def _build():
    import concourse.tile as tile
    from concourse import bacc
    from concourse import mybir

    f32 = mybir.dt.float32
    f32r = mybir.dt.float32r
    bf16 = (
        mybir.dt.float16
        if os.environ.get("MIXEDFFN_NS_DT", "fp16") == "fp16"
        else mybir.dt.bfloat16
    )
    ns_mode = os.environ.get("MIXEDFFN_NS", "mm2")
    ns1dt = bf16 if ns_mode in ("all", "mm1") else f32r
    ns2dt = bf16 if ns_mode in ("all", "mm2") else f32r
    GELU = (
        mybir.ActivationFunctionType.Relu
        if os.environ.get("MIXEDFFN_SIM_ACT") == "relu"
        else mybir.ActivationFunctionType.Gelu
    )

    nc = bacc.Bacc("TRN2", target_bir_lowering=False, debug=False, num_devices=NCORES)

    XT = nc.dram_tensor("XT", [NRB, 128, DC, RB], f32r, kind="ExternalInput").ap()
    PB = 16  # padded batch stride for 32B-aligned bf16 stationary slices
    XNS = nc.dram_tensor("XNS", [128, DC * PPC * PB], ns1dt, kind="ExternalInput").ap()
    W1T = nc.dram_tensor("W1T", [FC, 128, DC, 128], f32r, kind="ExternalInput").ap()
    W2T = nc.dram_tensor("W2T", [F, D], f32r, kind="ExternalInput").ap()
    W1N = nc.dram_tensor("W1N", [PPC, 2, DC, 128, F // 2], ns1dt, kind="ExternalInput").ap()
    W2N = nc.dram_tensor("W2N", [PPC, F, D], ns2dt, kind="ExternalInput").ap()
    IDEN = nc.dram_tensor("IDEN", [128, 128], f32r, kind="ExternalInput").ap()
    YS = nc.dram_tensor("YS", [S, D], f32, kind="ExternalOutput").ap()
    YN = nc.dram_tensor("YN", [PPC, B, D], f32, kind="ExternalOutput").ap()

    with tile.TileContext(nc) as tc:
        with (
            tc.tile_pool(name="wres", bufs=1) as wres,
            tc.tile_pool(name="xt", bufs=1) as xtp,
            tc.tile_pool(name="hth", bufs=1) as hthp,
            tc.tile_pool(name="ht01", bufs=2) as ht01p,
            tc.tile_pool(name="ysb", bufs=1) as ysbp,
            tc.tile_pool(name="w1n", bufs=int(os.environ.get("MIXEDFFN_W1N_BUFS", "6")) if ns1dt == bf16 else 3) as w1np,
            tc.tile_pool(name="w2n", bufs=(6 if ns2dt == bf16 else 3)) as w2np,
            tc.tile_pool(name="hns", bufs=1) as hnsp,
            tc.tile_pool(name="htns", bufs=1) as htnsp,
            tc.tile_pool(name="ph", bufs=2, space="PSUM") as php,
            tc.tile_pool(name="py", bufs=2, space="PSUM") as pyp,
            tc.tile_pool(name="nsps", bufs=1, space="PSUM") as nsps,
        ):
            # ---- resident tiles; weight loads are emitted as interleaved
            # steps so the first matmuls start after ~1MB, not 16MB ----
            w1t_sb = wres.tile([128, FC, DC, 128], f32r)
            w2t_sb = wres.tile([128, FC, D], f32r)
            xns_sb = wres.tile([128, DC * PPC * PB], ns1dt)
            nc.sync.dma_start(out=xns_sb[:], in_=XNS[:])
            ident = wres.tile([128, 128], f32r)
            nc.sync.dma_start(out=ident[:], in_=IDEN[:])

            def w_load_steps():
                def w1_step(fc):
                    def step():
                        nc.sync.dma_start(out=w1t_sb[:, fc], in_=W1T[fc])

                    return step

                def w2_step(fc):
                    def step():
                        nc.sync.dma_start(
                            out=w2t_sb[:, fc, :],
                            in_=W2T[fc * 128 : (fc + 1) * 128, :],
                        )

                    return step

                out = []
                for fc in range(FC):
                    out.append(w1_step(fc))
                    out.append(w2_step(fc))
                return out

            # ---- step generators ----
            state = {}

            def xt_load(rb):
                def step():
                    xt = xtp.tile([128, DC, RB], f32r, name=f"xtt{rb}", tag="xtt")
                    nc.sync.dma_start(out=xt[:], in_=XT[rb])
                    state[("xt", rb)] = xt

                return step

            def shared_steps(rb):
                steps = [xt_load(rb)] if rb == 0 else []

                def fc_step(fc):
                    def step():
                        xt = state[("xt", rb)]
                        if fc == 0:
                            state[("hth", rb)] = hthp.tile(
                                [128, FC, 256], f32r, name=f"hth{rb}", tag="hth"
                            )
                            state["py0"] = pyp.tile(
                                [128, D], f32, name=f"py{rb}_0", tag="py"
                            )
                            state["py1"] = pyp.tile(
                                [128, D], f32, name=f"py{rb}_1", tag="py"
                            )
                        hth = state[("hth", rb)]
                        ph = php.tile([128, RB], f32, name=f"ph{rb}_{fc}", tag="ph")
                        for dc in range(DC):
                            nc.tensor.matmul(
                                ph[:],
                                w1t_sb[:, fc, dc, :],
                                xt[:, dc, :],
                                start=(dc == 0),
                                stop=(dc == DC - 1),
                                skip_group_check=True,
                            )
                        ht01 = ht01p.tile(
                            [128, 256], f32r, name=f"ht01_{rb}_{fc}", tag="ht01"
                        )
                        nc.scalar.activation(ht01[:], ph[:, 0:256], GELU)
                        nc.scalar.activation(hth[:, fc, :], ph[:, 256:512], GELU)
                        for rc in range(2):
                            py = state[f"py{rc}"]
                            for dh in range(2):
                                nc.tensor.matmul(
                                    py[:, dh * 512 : (dh + 1) * 512],
                                    ht01[:, rc * 128 : (rc + 1) * 128],
                                    w2t_sb[:, fc, dh * 512 : (dh + 1) * 512],
                                    start=(fc == 0),
                                    stop=(fc == FC - 1),
                                    skip_group_check=True,
                                )

                    return step

                def y01_step(rc):
                    def step():
                        py = state[f"py{rc}"]
                        ysb = ysbp.tile(
                            [128, D], f32, name=f"ysb{rb}_{rc}", tag="ysb"
                        )
                        nc.vector.tensor_copy(ysb[:], py[:])
                        row0 = rb * RB + rc * 128
                        nrows = min(128, max(0, S - row0))
                        if nrows > 0:
                            nc.sync.dma_start(
                                out=YS[row0 : row0 + nrows, :], in_=ysb[:nrows, :]
                            )

                    return step

                def mm2b_step(rc):
                    def step():
                        hth = state[("hth", rb)]
                        py = pyp.tile([128, D], f32, name=f"py{rb}_{rc}", tag="py")
                        for dh in range(2):
                            for fc in range(FC):
                                nc.tensor.matmul(
                                    py[:, dh * 512 : (dh + 1) * 512],
                                    hth[:, fc, (rc - 2) * 128 : (rc - 1) * 128],
                                    w2t_sb[:, fc, dh * 512 : (dh + 1) * 512],
                                    start=(fc == 0),
                                    stop=(fc == FC - 1),
                                    skip_group_check=True,
                                )
                        ysb = ysbp.tile(
                            [128, D], f32, name=f"ysb{rb}_{rc}", tag="ysb"
                        )
                        nc.vector.tensor_copy(ysb[:], py[:])
                        row0 = rb * RB + rc * 128
                        nrows = min(128, max(0, S - row0))
                        if nrows > 0:
                            nc.sync.dma_start(
                                out=YS[row0 : row0 + nrows, :], in_=ysb[:nrows, :]
                            )

                    return step

                steps += [fc_step(fc) for fc in range(FC)]
                if rb + 1 < NRB:
                    steps.append(xt_load(rb + 1))
                steps += [y01_step(0), y01_step(1), mm2b_step(2), mm2b_step(3)]
                return steps

            def ns_steps(p):
                """50 steps for one NS position: 32 MM1 (fb,dc) + 2 transpose
                batches + 16 MM2 (fc)."""
                steps = []

                def mm1_step(half, dc):
                    def step():
                        if half == 0 and dc == 0:
                            state["hsb"] = hnsp.tile(
                                [B, F], f32r, name=f"hsb{p}", tag="hsb"
                            )
                        if dc == 0:
                            state["phn"] = nsps.tile(
                                [B, F // 2], f32, name=f"phn{p}_{half}", tag="nsps"
                            )
                        phn = state["phn"]
                        w1 = w1np.tile(
                            [128, F // 2], ns1dt, name=f"w1_{p}_{half}_{dc}", tag="w1"
                        )
                        nc.sync.dma_start(out=w1[:], in_=W1N[p, half, dc])
                        for fbh in range(2):
                            nc.tensor.matmul(
                                phn[:, fbh * 512 : (fbh + 1) * 512],
                                xns_sb[
                                    :,
                                    dc * PPC * PB
                                    + p * PB : dc * PPC * PB
                                    + p * PB
                                    + B,
                                ],
                                w1[:, fbh * 512 : (fbh + 1) * 512],
                                start=(dc == 0),
                                stop=(dc == DC - 1),
                            )
                        if dc == DC - 1:
                            nc.scalar.activation(
                                state["hsb"][
                                    :, half * (F // 2) : (half + 1) * (F // 2)
                                ],
                                phn[:],
                                GELU,
                            )

                    return step

                def tr_step(pair):
                    def step():
                        hsb = state["hsb"]
                        if pair == 0:
                            state["hTns"] = htnsp.tile(
                                [128, FC, B], ns2dt, name=f"hT{p}", tag="hT"
                            )
                        hT = state["hTns"]
                        for c in range(pair * 2, pair * 2 + 2):
                            pt = nsps.tile(
                                [128, B], f32r, name=f"pt{p}_{c}", tag="nsps"
                            )
                            nc.tensor.transpose(
                                pt[:], hsb[0:B, c * 128 : (c + 1) * 128], ident[0:B, 0:B]
                            )
                            nc.vector.tensor_copy(hT[:, c, :], pt[:])

                    return step

                def mm2_step(fc):
                    def step():
                        if fc == 0:
                            state["pyn"] = nsps.tile(
                                [B, D], f32, name=f"pyn{p}", tag="nsps"
                            )
                        pyn = state["pyn"]
                        hT = state["hTns"]
                        w2 = w2np.tile([128, D], ns2dt, name=f"w2_{p}_{fc}", tag="w2")
                        nc.sync.dma_start(
                            out=w2[:], in_=W2N[p, fc * 128 : (fc + 1) * 128, :]
                        )
                        for dh in range(2):
                            nc.tensor.matmul(
                                pyn[:, dh * 512 : (dh + 1) * 512],
                                hT[:, fc, :],
                                w2[:, dh * 512 : (dh + 1) * 512],
                                start=(fc == 0),
                                stop=(fc == FC - 1),
                            )
                        if fc == FC - 1:
                            ysb = ysbp.tile([B, D], f32, name=f"ysbn{p}", tag="ysb")
                            nc.vector.tensor_copy(ysb[:], pyn[:])
                            nc.sync.dma_start(out=YN[p], in_=ysb[:])

                    return step

                steps += [mm1_step(half, dc) for half in range(2) for dc in range(DC)]
                steps += [tr_step(pair) for pair in range(8)]
                steps += [mm2_step(fc) for fc in range(FC)]
                return steps

            # ---- emission: block 0 carries the W-loads and NS position 0;
            # later blocks carry two NS positions each ----
            # Flatten all NS steps and distribute across blocks so each
            # block's total DMA (shared + NS share) is roughly equal.
            all_ns = [st for p in range(LNS // NCORES) for st in ns_steps(p)]
            nsplit = [40, 90, 90, 100]
            assert sum(nsplit) == len(all_ns), (len(all_ns), nsplit)
            ns_off = [0]
            for c in nsplit:
                ns_off.append(ns_off[-1] + c)
            for rb in range(NRB):
                sh = shared_steps(rb)
                if rb == 0:
                    # Weave the per-fc weight loads ahead of their consumers
                    # (program order IS the dependency order under Tile: a
                    # consumer emitted before its producer reads stale data).
                    wl = w_load_steps()  # [w1(0), w2(0), w1(1), w2(1), ...]
                    woven = [sh[0]] + wl[0:4]
                    rest = sh[1:]
                    for k, st in enumerate(rest):
                        woven.append(st)
                        lo, hi = 4 + 2 * k, 4 + 2 * (k + 1)
                        woven += wl[lo:hi]
                    sh = woven
                nsl = all_ns[ns_off[rb] : ns_off[rb + 1]]
                for st in _interleave(sh, nsl):
                    st()

    nc.compile()
    return nc


def _prepare_inputs(x, W1S, W2S, W1NS, W2NS):
    x = np.ascontiguousarray(x, dtype=np.float32)
    # [FC, 128, DC, 128] per-fc contiguous blocks of W1S.T
    w1t = np.ascontiguousarray(
        np.asarray(W1S.T, dtype=np.float32)
        .reshape(DC, 128, FC, 128)
        .transpose(2, 1, 0, 3)
    )
    w2t = np.ascontiguousarray(W2S.T, dtype=np.float32)  # [F, D]
    iden = np.eye(128, dtype=np.float32)
    in_maps = []
    for i in range(NCORES):
        # [NRB, 128, DC, RB]: per row-block, partition-major, 16KB runs
        xt = np.ascontiguousarray(
            x[i].T.reshape(DC, 128, NRB, RB).transpose(2, 1, 0, 3)
        )
        xi = x[:, S + PPC * i : S + PPC * (i + 1), :]  # [B, PPC, D]
        # [128, dc, p, b] flattened to [128, dc*p*b]
        xns4 = (
            xi.transpose(2, 1, 0)  # [D, PPC, B]
            .reshape(DC, 128, PPC, B)
            .transpose(1, 0, 2, 3)
            .astype(_ns1_np())
        )  # [128, DC, PPC, B]
        xns = np.zeros((128, DC, PPC, 16), dtype=xns4.dtype)
        xns[:, :, :, :B] = xns4
        xns = np.ascontiguousarray(xns.reshape(128, DC * PPC * 16))
        in_maps.append(
            {
                "XT": xt,
                "XNS": xns,
                "W1T": w1t,
                "W2T": w2t,
                "IDEN": iden,
                "W1N": np.ascontiguousarray(
                    W1NS[PPC * i : PPC * (i + 1)]
                    .reshape(PPC, DC, 128, 2, F // 2)
                    .transpose(0, 3, 1, 2, 4)
                    .astype(_ns1_np())
                ),
                "W2N": np.ascontiguousarray(W2NS[PPC * i : PPC * (i + 1)]).astype(_ns2_np()),
            }
        )
    return in_maps


def kernel(x, W1S, W2S, W1NS, W2NS):
    global _cached, LAST_RESULTS
    from concourse.bass_utils import run_bass_kernel_spmd

    if _cached is None:
        _cached = _build()
    nc = _cached
    in_maps = _prepare_inputs(x, W1S, W2S, W1NS, W2NS)
    trace = bool(os.environ.get("MIXEDFFN_TRACE"))
    res = run_bass_kernel_spmd(
        nc, in_maps, core_ids=list(range(NCORES)), trace=trace
    )
    LAST_RESULTS = res
    out = np.empty((B, T, D), dtype=np.float32)
    for i in range(NCORES):
        out[i, :S, :] = res.results[i]["YS"]
        yn = res.results[i]["YN"]  # [PPC, B, D]
        for p in range(PPC):
            out[:, S + PPC * i + p, :] = yn[p]
    return out



# revision 2
# speedup vs baseline: 1.4114x; 1.4114x over previous
"""MixedFFN Trainium2 kernel (8 NeuronCores, SPMD).

Problem: x [8, 2048, 1024]; shared FFN (W1S [2048,1024], W2S [1024,2048])
applied to positions 0..1984 of every batch; per-position FFN
(W1NS [64,1024,2048], W2NS [64,2048,1024]) applied to positions 1984..2048.
gelu is exact (erf). Output [8, 2048, 1024] fp32.

Sharding:
  - Shared part: data-parallel over batch. Core i computes the shared FFN
    for batch i over the 1984 shared positions using replicated W1S/W2S.
  - Per-position part: sharded over positions. Core i handles positions
    1984+8i .. 1984+8(i+1) for ALL batches, so each NS weight byte is read
    from HBM exactly once across the chip.

Device kernel (per core, identical program, different data):
  - All matmul operands are fp16 (full-rate PE, half the HBM bytes of
    fp32; K<=2048 contractions accumulate in fp32 PSUM so rel err ~1e-3).
  - Shared block rb (512/512/512/448 rows): MM1 phase (per fc: 8 dc-chunk
    matmuls -> PSUM, gelu -> hth[:, fc] fp16), then MM2 phase (per rc,dh:
    16 fc matmuls from hth -> PSUM, copy, DMA out). Phases are dense
    back-to-back PE work so the tensor engine stays at its top DVFS state.
  - NS position p: 8 resident W1N dc-tiles [128, 2048] (one DMA each,
    4KB lines), 4 quarter accumulations [8,512] over dc, gelu -> hsb,
    16 PE transposes -> hT fp16, then 8 fc-pair W2N tiles [128, 2048]
    with 4 matmuls each accumulating y [8,1024].
  - NS steps are Bresenham-interleaved with shared steps so the NS weight
    stream (64MB/core, the DMA bulk) overlaps shared compute, while PSUM
    fits: ph 2 + py 2 + nsq 2 + pyn 2 = 8 banks.

Host side: shards/casts/packs inputs (numpy), feeds the SPMD run,
reassembles.
"""

import os
import sys

import numpy as np

for _p in ("/opt/trn_rl_repo",):
    if os.path.isdir(_p) and _p not in sys.path:
        sys.path.insert(0, _p)

B, T, D, F, LNS = 8, 2048, 1024, 2048, 64
S = T - LNS  # 1984
NCORES = 8
PPC = LNS // NCORES  # 8 positions per core
DC, FC = D // 128, F // 128  # 8, 16 k-chunks
NRB = 4
RBS = [512, 512, 512, 448]  # row-block sizes covering the S=1984 rows
RBOFF = [0, 512, 1024, 1536]
XTLEN = DC * S  # free length of the packed x tensor
PB = 16  # padded batch stride for 32B-aligned fp16 stationary slices

LAST_RESULTS = None  # BassKernelResults of the most recent run (for test.py)

_cached = None


def _interleave(a, b):
    """Merge two step lists proportionally (Bresenham); each step is a
    zero-arg callable that emits instructions."""
    if not b:
        return list(a)
    if not a:
        return list(b)
    out = []
    ia = ib = 0
    na, nb = len(a), len(b)
    while ia < na or ib < nb:
        if ib * na <= ia * nb:
            if ib < nb:
                out.append(b[ib])
                ib += 1
            else:
                out.append(a[ia])
                ia += 1
        else:
            if ia < na:
                out.append(a[ia])
                ia += 1
            else:
                out.append(b[ib])
                ib += 1
    return out


def _build():
    import concourse.tile as tile
    from concourse import bacc
    from concourse import mybir

    f32 = mybir.dt.float32
    f32r = mybir.dt.float32r
    f16 = mybir.dt.float16
    GELU = mybir.ActivationFunctionType.Gelu
    W1N_BUFS = int(os.environ.get("MIXEDFFN_W1N_BUFS", "12"))
    W2N_BUFS = int(os.environ.get("MIXEDFFN_W2N_BUFS", "6"))

    nc = bacc.Bacc("TRN2", target_bir_lowering=False, debug=False, num_devices=NCORES)

    XT = nc.dram_tensor("XT", [128, XTLEN], f16, kind="ExternalInput").ap()
    XNS = nc.dram_tensor("XNS", [128, DC * PPC * PB], f16, kind="ExternalInput").ap()
    W1TP = nc.dram_tensor("W1TP", [FC // 2, 128, 2 * DC * 128], f16, kind="ExternalInput").ap()
    W2TP = nc.dram_tensor("W2TP", [FC // 2, 128, 2 * D], f16, kind="ExternalInput").ap()
    W1N = nc.dram_tensor("W1N", [PPC, D, F], f16, kind="ExternalInput").ap()
    W2NP = nc.dram_tensor("W2NP", [PPC, FC // 2, 128, 2 * D], f16, kind="ExternalInput").ap()
    IDEN = nc.dram_tensor("IDEN", [128, 128], f32r, kind="ExternalInput").ap()
    YS = nc.dram_tensor("YS", [S, D], f32, kind="ExternalOutput").ap()
    YN = nc.dram_tensor("YN", [PPC, B, D], f32, kind="ExternalOutput").ap()

    with tile.TileContext(nc) as tc:
        with (
            tc.tile_pool(name="wres", bufs=1) as wres,
            tc.tile_pool(name="xt", bufs=2) as xtp,
            tc.tile_pool(name="hth", bufs=1) as hthp,
            tc.tile_pool(name="ysb", bufs=2) as ysbp,
            tc.tile_pool(name="w1n", bufs=W1N_BUFS) as w1np,
            tc.tile_pool(name="w2n", bufs=W2N_BUFS) as w2np,
            tc.tile_pool(name="hns", bufs=1) as hnsp,
            tc.tile_pool(name="htns", bufs=1) as htnsp,
            tc.tile_pool(name="ph", bufs=2, space="PSUM") as php,
            tc.tile_pool(name="py", bufs=2, space="PSUM") as pyp,
            tc.tile_pool(name="nsq", bufs=2, space="PSUM") as nsqp,
            tc.tile_pool(name="pyn", bufs=1, space="PSUM") as pynp,
        ):
            # ---- resident tiles; weight loads are emitted as interleaved
            # steps so the first matmuls start after ~1MB, not 8MB ----
            w1t_sb = wres.tile([128, FC, DC, 128], f16)
            w2t_sb = wres.tile([128, FC, D], f16)
            xns_sb = wres.tile([128, DC * PPC * PB], f16)
            nc.sync.dma_start(out=xns_sb[:], in_=XNS[:])
            ident = wres.tile([128, 128], f32r)
            nc.sync.dma_start(out=ident[:], in_=IDEN[:])

            def w_load_steps():
                def w1_step(j):
                    def step():
                        nc.sync.dma_start(
                            out=w1t_sb[:, 2 * j : 2 * j + 2], in_=W1TP[j]
                        )

                    return step

                def w2_step(j):
                    def step():
                        nc.sync.dma_start(
                            out=w2t_sb[:, 2 * j : 2 * j + 2, :], in_=W2TP[j]
                        )

                    return step

                out = []
                for j in range(FC // 2):
                    out.append(w1_step(j))
                    out.append(w2_step(j))
                return out

            # ---- step generators ----
            state = {}

            def xt_load(rb):
                def step():
                    rbsz = RBS[rb]
                    xt = xtp.tile([128, DC, rbsz], f16, name=f"xtt{rb}", tag="xtt")
                    off = DC * RBOFF[rb]
                    nc.sync.dma_start(out=xt[:], in_=XT[:, off : off + DC * rbsz])
                    state[("xt", rb)] = xt

                return step

            def shared_steps(rb):
                rbsz = RBS[rb]
                steps = [xt_load(rb)] if rb == 0 else []

                def mm1_step(fc):
                    def step():
                        xt = state[("xt", rb)]
                        if fc == 0:
                            state[("hth", rb)] = hthp.tile(
                                [128, FC, rbsz], f16, name=f"hth{rb}", tag="hth"
                            )
                        hth = state[("hth", rb)]
                        ph = php.tile([128, rbsz], f32, name=f"ph{rb}_{fc}", tag="ph")
                        for dc in range(DC):
                            nc.tensor.matmul(
                                ph[:],
                                w1t_sb[:, fc, dc, :],
                                xt[:, dc, :],
                                start=(dc == 0),
                                stop=(dc == DC - 1),
                                skip_group_check=True,
                            )
                        nc.scalar.activation(hth[:, fc, :], ph[:], GELU)

                    return step

                def mm2_step(rc, dh):
                    def step():
                        hth = state[("hth", rb)]
                        nrows = min(128, rbsz - rc * 128)
                        py = pyp.tile(
                            [nrows, 512], f32, name=f"py{rb}_{rc}_{dh}", tag="py"
                        )
                        for fc in range(FC):
                            nc.tensor.matmul(
                                py[:],
                                hth[:, fc, rc * 128 : rc * 128 + nrows],
                                w2t_sb[:, fc, dh * 512 : (dh + 1) * 512],
                                start=(fc == 0),
                                stop=(fc == FC - 1),
                                skip_group_check=True,
                            )
                        if dh == 0:
                            state[("ysb", rb, rc)] = ysbp.tile(
                                [nrows, D], f32, name=f"ysb{rb}_{rc}", tag="ysb"
                            )
                        ysb = state[("ysb", rb, rc)]
                        nc.vector.tensor_copy(
                            ysb[:, dh * 512 : (dh + 1) * 512], py[:]
                        )
                        if dh == 1:
                            row0 = RBOFF[rb] + rc * 128
                            nc.sync.dma_start(
                                out=YS[row0 : row0 + nrows, :], in_=ysb[:]
                            )

                    return step

                steps += [mm1_step(fc) for fc in range(FC)]
                if rb + 1 < NRB:
                    steps.insert(len(steps) // 2, xt_load(rb + 1))
                nrc = (rbsz + 127) // 128
                steps += [mm2_step(rc, dh) for rc in range(nrc) for dh in range(2)]
                return steps

            def ns_steps(p):
                """Steps for one NS position: 8 W1 loads + 4 quarter-MM1 +
                8 transpose pairs + 8 fc-pair MM2."""
                steps = []

                def w1_load(dc):
                    def step():
                        w1 = w1np.tile(
                            [128, F], f16, name=f"w1_{p}_{dc}", tag="w1"
                        )
                        nc.sync.dma_start(
                            out=w1[:], in_=W1N[p, dc * 128 : (dc + 1) * 128, :]
                        )
                        state[("w1n", dc)] = w1

                    return step

                def mm1_step(q):
                    def step():
                        if q == 0:
                            state["hsb"] = hnsp.tile(
                                [B, F], f32r, name=f"hsb{p}", tag="hsb"
                            )
                        phq = nsqp.tile(
                            [B, 512], f32, name=f"phq{p}_{q}", tag="nsq"
                        )
                        for dc in range(DC):
                            nc.tensor.matmul(
                                phq[:],
                                xns_sb[
                                    :,
                                    dc * PPC * PB
                                    + p * PB : dc * PPC * PB
                                    + p * PB
                                    + B,
                                ],
                                state[("w1n", dc)][:, q * 512 : (q + 1) * 512],
                                start=(dc == 0),
                                stop=(dc == DC - 1),
                                skip_group_check=True,
                            )
                        nc.scalar.activation(
                            state["hsb"][:, q * 512 : (q + 1) * 512], phq[:], GELU
                        )

                    return step

                def tr_step(pair):
                    def step():
                        hsb = state["hsb"]
                        if pair == 0:
                            state["hTns"] = htnsp.tile(
                                [128, FC, B], f16, name=f"hT{p}", tag="hT"
                            )
                        hT = state["hTns"]
                        for c in range(pair * 2, pair * 2 + 2):
                            pt = nsqp.tile(
                                [128, B], f32r, name=f"pt{p}_{c}", tag="nsq"
                            )
                            nc.tensor.transpose(
                                pt[:], hsb[0:B, c * 128 : (c + 1) * 128], ident[0:B, 0:B]
                            )
                            nc.vector.tensor_copy(hT[:, c, :], pt[:])

                    return step

                def mm2_step(j):
                    def step():
                        if j == 0:
                            state["pyn"] = pynp.tile(
                                [B, D], f32, name=f"pyn{p}", tag="pyn"
                            )
                        pyn = state["pyn"]
                        hT = state["hTns"]
                        w2 = w2np.tile([128, 2 * D], f16, name=f"w2_{p}_{j}", tag="w2")
                        nc.sync.dma_start(out=w2[:], in_=W2NP[p, j])
                        for jj in range(2):
                            fc = 2 * j + jj
                            for dh in range(2):
                                nc.tensor.matmul(
                                    pyn[:, dh * 512 : (dh + 1) * 512],
                                    hT[:, fc, :],
                                    w2[:, jj * D + dh * 512 : jj * D + (dh + 1) * 512],
                                    start=(fc == 0),
                                    stop=(fc == FC - 1),
                                    skip_group_check=True,
                                )
                        if j == FC // 2 - 1:
                            ysb = ysbp.tile([B, D], f32, name=f"ysbn{p}", tag="ysb")
                            nc.vector.tensor_copy(ysb[:], pyn[:])
                            nc.sync.dma_start(out=YN[p], in_=ysb[:])

                    return step

                steps += [w1_load(dc) for dc in range(DC)]
                steps += [mm1_step(q) for q in range(4)]
                steps += [tr_step(pair) for pair in range(8)]
                steps += [mm2_step(j) for j in range(FC // 2)]
                return steps

            # ---- emission: block 0 carries the W-resident loads; NS steps
            # are distributed across blocks so each block's DMA load
            # (shared + NS share) is roughly equal ----
            all_ns = [st for p in range(PPC) for st in ns_steps(p)]
            nsplit = [35, 66, 66, 57]
            assert sum(nsplit) == len(all_ns), (len(all_ns), nsplit)
            ns_off = [0]
            for c in nsplit:
                ns_off.append(ns_off[-1] + c)
            for rb in range(NRB):
                sh = shared_steps(rb)
                if rb == 0:
                    # Weave the per-pair weight loads ahead of their consumers
                    # (program order IS the dependency order under Tile: a
                    # consumer emitted before its producer reads stale data).
                    wl = w_load_steps()  # [w1(0), w2(0), w1(1), w2(1), ...]
                    woven = [sh[0]] + wl[0:4]
                    rest = sh[1:]
                    for k, st in enumerate(rest):
                        woven.append(st)
                        lo, hi = 4 + 2 * k, 4 + 2 * (k + 1)
                        woven += wl[lo:hi]
                    sh = woven
                nsl = all_ns[ns_off[rb] : ns_off[rb + 1]]
                for st in _interleave(sh, nsl):
                    st()

    nc.compile()
    return nc


def _prepare_inputs(x, W1S, W2S, W1NS, W2NS):
    x = np.asarray(x, dtype=np.float32)
    # W1TP [FC//2, 128, 2, DC, 128]: fc-pair-packed blocks of W1S.T
    w1t = (
        np.asarray(W1S.T, dtype=np.float16)
        .reshape(DC, 128, FC, 128)
        .transpose(2, 1, 0, 3)
    )  # [FC, 128, DC, 128]
    w1tp = np.ascontiguousarray(
        w1t.reshape(FC // 2, 2, 128, DC, 128)
        .transpose(0, 2, 1, 3, 4)
        .reshape(FC // 2, 128, 2 * DC * 128)
    )
    # W2TP [FC//2, 128, 2*D]: fc-pair-packed chunks of W2S.T
    w2t = np.asarray(W2S.T, dtype=np.float16).reshape(FC // 2, 2, 128, D)
    w2tp = np.ascontiguousarray(
        w2t.transpose(0, 2, 1, 3).reshape(FC // 2, 128, 2 * D)
    )
    iden = np.eye(128, dtype=np.float32)
    in_maps = []
    for i in range(NCORES):
        # XT [128, DC*S]: concatenated row blocks, partition-major
        xt = np.empty((128, XTLEN), dtype=np.float16)
        for rb in range(NRB):
            rbsz = RBS[rb]
            blk = (
                x[i][RBOFF[rb] : RBOFF[rb] + rbsz, :]
                .T.reshape(DC, 128, rbsz)
                .transpose(1, 0, 2)
                .reshape(128, DC * rbsz)
            )
            off = DC * RBOFF[rb]
            xt[:, off : off + DC * rbsz] = blk
        xi = x[:, S + PPC * i : S + PPC * (i + 1), :]  # [B, PPC, D]
        # [128, dc, p, b] flattened to [128, dc*p*PB]
        xns4 = (
            xi.transpose(2, 1, 0)  # [D, PPC, B]
            .reshape(DC, 128, PPC, B)
            .transpose(1, 0, 2, 3)
            .astype(np.float16)
        )  # [128, DC, PPC, B]
        xns = np.zeros((128, DC, PPC, PB), dtype=np.float16)
        xns[:, :, :, :B] = xns4
        xns = np.ascontiguousarray(xns.reshape(128, DC * PPC * PB))
        w2n = (
            np.asarray(W2NS[PPC * i : PPC * (i + 1)], dtype=np.float16)
            .reshape(PPC, FC // 2, 2, 128, D)
            .transpose(0, 1, 3, 2, 4)
            .reshape(PPC, FC // 2, 128, 2 * D)
        )
        in_maps.append(
            {
                "XT": np.ascontiguousarray(xt),
                "XNS": xns,
                "W1TP": w1tp,
                "W2TP": w2tp,
                "IDEN": iden,
                "W1N": np.ascontiguousarray(
                    W1NS[PPC * i : PPC * (i + 1)].astype(np.float16)
                ),
                "W2NP": np.ascontiguousarray(w2n),
            }
        )
    return in_maps


def kernel(x, W1S, W2S, W1NS, W2NS):
    global _cached, LAST_RESULTS
    from concourse.bass_utils import run_bass_kernel_spmd

    if _cached is None:
        _cached = _build()
    nc = _cached
    in_maps = _prepare_inputs(x, W1S, W2S, W1NS, W2NS)
    trace = bool(os.environ.get("MIXEDFFN_TRACE"))
    res = run_bass_kernel_spmd(
        nc, in_maps, core_ids=list(range(NCORES)), trace=trace
    )
    LAST_RESULTS = res
    out = np.empty((B, T, D), dtype=np.float32)
    for i in range(NCORES):
        out[i, :S, :] = res.results[i]["YS"]
        yn = res.results[i]["YN"]  # [PPC, B, D]
        for p in range(PPC):
            out[:, S + PPC * i + p, :] = yn[p]
    return out
